# revision 8
# baseline (speedup 1.0000x reference)
"""Trainium2 Bass kernel for nn_CWVAE: 3-level clockwork VAE (GRU hierarchy).

Strategy (8 cores, data-parallel over batch B=32 -> b=4 rows/core):
  - Everything on-chip runs in *transposed* layout: [feature(128-part), qtile, cols]
    so matmuls keep weights stationary (bf16 + fast-weight-load) and batch moves.
  - Per level (top->bottom):
      obs_pre = W_obs[512:]^T @ obs^T + b_obs      (big GEMM, DRAM-staged)
      ctxb    = W_in[64:]^T @ det_{l+1}^T + b_in   (GEMM, SBUF-resident)
      sequential GRU scan over T_l steps; only the deterministic state (and the
      posterior mean chain feeding the next step's sample) is computed --
      prior/posterior std outputs of the reference are dead code.
  - Host pre-transposes obs / pre-tiles weights (bf16) and un-transposes the
    final det output, so the kernel does zero on-chip transposes.
"""

import numpy as np
import ml_dtypes

import concourse.bass as bass
import concourse.bacc as bacc
import concourse.tile as tile
from concourse import mybir
from concourse.bass import ds
from concourse.tile_rust import add_dep_helper
from concourse.bass_utils import run_bass_kernel_spmd

BF16 = mybir.dt.bfloat16
F32 = mybir.dt.float32
AF = mybir.ActivationFunctionType
ALU = mybir.AluOpType

LEVELS = 3
FACTOR = 4
B = 32
T0 = 1024
STOCH = 64
DETER = 512
EMBED = 512
OBS_EMBED = 1024
NCORES = 8
BPC = B // NCORES          # batch rows per core = 4
TS = [T0, T0 // 4, T0 // 16]   # per-level sequence length, index by level
U = 32                     # scan steps unrolled per For_i iteration
GN = 512                   # GEMM chunk (columns of t*b)


def _q(n):  # number of 128-partition tiles in n
    return n // 128


class _DepChain:
    """Serialize matmul accumulation groups that share a PSUM bank (start=True
    clears has_written for the whole bank, so groups must not interleave)."""

    def __init__(self):
        self.last = {}

    def mm(self, nc, tag, out, lhsT, rhs, start, stop):
        inst = nc.tensor.matmul(out, lhsT, rhs, start=start, stop=stop)
        if start and tag in self.last:
            add_dep_helper(inst.ins, self.last[tag], reason="psum group order")
        if stop:
            self.last[tag] = inst.ins
        return inst


def build_nc(do_gemm=True, do_scan=True, levels=(2, 1, 0), do_ctx=None):
    nc = bacc.Bacc("TRN2", target_bir_lowering=False)
    b = BPC

    # ---------------- DRAM tensors ----------------
    obsT = [nc.dram_tensor(f"obsT{l}", [_q(OBS_EMBED), 128, TS[l] * b], BF16,
                           kind="ExternalInput") for l in range(LEVELS)]
    wis = [nc.dram_tensor(f"wis{l}", [STOCH, EMBED], BF16, kind="ExternalInput")
           for l in range(LEVELS)]
    wih = [nc.dram_tensor(f"wih{l}", [4, 128, 3 * DETER], BF16, kind="ExternalInput")
           for l in range(LEVELS)]
    whh = [nc.dram_tensor(f"whh{l}", [4, 128, 3 * DETER], BF16, kind="ExternalInput")
           for l in range(LEVELS)]
    woh = [nc.dram_tensor(f"woh{l}", [4, 128, DETER], BF16, kind="ExternalInput")
           for l in range(LEVELS)]
    wpm = [nc.dram_tensor(f"wpm{l}", [4, 128, STOCH], BF16, kind="ExternalInput")
           for l in range(LEVELS)]
    woo = [nc.dram_tensor(f"woo{l}", [8, 128, DETER], BF16, kind="ExternalInput")
           for l in range(LEVELS)]
    wic = [nc.dram_tensor(f"wic{l}", [4, 128, EMBED], BF16, kind="ExternalInput")
           for l in range(2)]
    brz = [nc.dram_tensor(f"brz{l}", [128, 8, b], F32, kind="ExternalInput")
           for l in range(LEVELS)]
    bhn = [nc.dram_tensor(f"bhn{l}", [128, 4, b], F32, kind="ExternalInput")
           for l in range(LEVELS)]
    bin_n = [nc.dram_tensor(f"binn{l}", [128, 4, b], F32, kind="ExternalInput")
             for l in range(LEVELS)]
    bpost = [nc.dram_tensor(f"bpost{l}", [STOCH, 1], F32, kind="ExternalInput")
             for l in range(LEVELS)]
    bobs = [nc.dram_tensor(f"bobs{l}", [128, 4], F32, kind="ExternalInput")
            for l in range(LEVELS)]
    binc = [nc.dram_tensor(f"binc{l}", [128, 4], F32, kind="ExternalInput")
            for l in range(2)]
    cbtop = nc.dram_tensor("cbtop", [128, 4, b], BF16, kind="ExternalInput")

    obspre = [nc.dram_tensor(f"obspre{l}", [4, 128, TS[l] * b], BF16, kind="Internal")
              for l in range(LEVELS)]
    det0T = nc.dram_tensor("det0T", [4, 128, T0 * b], F32, kind="ExternalOutput")

    with tile.TileContext(nc) as tc:
        with (
            tc.tile_pool(name="wpool", bufs=1) as wpool,
            tc.tile_pool(name="state", bufs=1) as state,
            tc.tile_pool(name="gio", bufs=3) as gio,
            tc.tile_pool(name="work", bufs=2) as work,
            tc.tile_pool(name="opool", bufs=2) as opool,
            tc.tile_pool(name="psg", bufs=2, space="PSUM") as psg,
            tc.tile_pool(name="pss", bufs=1, space="PSUM") as pss,
        ):
            dep = _DepChain()

            # -------- load weights / biases to SBUF --------
            def load(dr, shape, dt, nm, re=None):
                t = wpool.tile(shape, dt, name=nm, tag=nm)
                src = dr[:, :, :] if len(dr.shape) == 3 else dr[:, :]
                if re:
                    src = src.rearrange(re)
                nc.sync.dma_start(out=t, in_=src)
                return t

            wis_s = [load(wis[l], [STOCH, EMBED], BF16, f"wis_s{l}") for l in range(LEVELS)]
            wih_s = [load(wih[l], [128, 4, 3 * DETER], BF16, f"wih_s{l}", "k p m -> p k m")
                     for l in range(LEVELS)]
            whh_s = [load(whh[l], [128, 4, 3 * DETER], BF16, f"whh_s{l}", "k p m -> p k m")
                     for l in range(LEVELS)]
            woh_s = [load(woh[l], [128, 4, DETER], BF16, f"woh_s{l}", "k p m -> p k m")
                     for l in range(LEVELS)]
            wpm_s = [load(wpm[l], [128, 4, STOCH], BF16, f"wpm_s{l}", "k p m -> p k m")
                     for l in range(LEVELS)]
            woo_s = [load(woo[l], [128, 8, DETER], BF16, f"woo_s{l}", "k p m -> p k m")
                     for l in range(LEVELS)]
            wic_s = [load(wic[l], [128, 4, EMBED], BF16, f"wic_s{l}", "k p m -> p k m")
                     for l in range(2)]
            brz_s = [load(brz[l], [128, 8, b], F32, f"brz_s{l}") for l in range(LEVELS)]
            bhn_s = [load(bhn[l], [128, 4, b], F32, f"bhn_s{l}") for l in range(LEVELS)]
            binn_s = [load(bin_n[l], [128, 4, b], F32, f"binn_s{l}") for l in range(LEVELS)]
            bpost_s = [load(bpost[l], [STOCH, 1], F32, f"bpost_s{l}") for l in range(LEVELS)]
            bobs_s = [load(bobs[l], [128, 4], F32, f"bobs_s{l}") for l in range(LEVELS)]
            binc_s = [load(binc[l], [128, 4], F32, f"binc_s{l}") for l in range(2)]
            cbtop_s = load(cbtop, [128, 4, b], BF16, "cbtop_s")

            # persistent per-level buffers
            det_sb = {k: state.tile([128, 4, TS[k] * b], BF16,
                                    name=f"det_sb{k}", tag=f"det_sb{k}")
                      for k in (1, 2)}
            ctxb_sb = {k: state.tile([128, 4, TS[k + 1] * b], BF16,
                                     name=f"ctxb_sb{k}", tag=f"ctxb_sb{k}")
                       for k in (0, 1)}

            # -------- phase helpers --------
            def obs_gemm(l):
                total = TS[l] * b
                nch = (total + GN - 1) // GN
                for c in range(nch):
                    n0, n1 = c * GN, min((c + 1) * GN, total)
                    n = n1 - n0
                    rhs = gio.tile([128, 8, GN], BF16, tag="gemm_rhs")
                    nc.sync.dma_start(
                        out=rhs[:, :, :n],
                        in_=obsT[l][:, :, n0:n1].rearrange("k p n -> p k n"))
                    for m in range(4):
                        ps = psg.tile([128, GN], F32, tag="gemm_ps")
                        for k in range(8):
                            dep.mm(nc, "gemm_ps", ps[:, :n],
                                   woo_s[l][:, k, 128 * m:128 * (m + 1)],
                                   rhs[:, k, :n], start=(k == 0), stop=(k == 7))
                        ob = gio.tile([128, GN], BF16, tag="gemm_out")
                        nc.vector.tensor_scalar_add(ob[:, :n], ps[:, :n],
                                                    bobs_s[l][:, m:m + 1])
                        nc.sync.dma_start(out=obspre[l][m, :, n0:n1], in_=ob[:, :n])

            def ctx_gemm(l):
                # ctxb_sb[l] = W_in_c[l]^T @ det_{l+1} + b_in  (bf16)
                total = TS[l + 1] * b
                nch = (total + GN - 1) // GN
                for c in range(nch):
                    n0, n1 = c * GN, min((c + 1) * GN, total)
                    n = n1 - n0
                    for m in range(4):
                        ps = psg.tile([128, GN], F32, tag="gemm_ps")
                        for k in range(4):
                            dep.mm(nc, "gemm_ps", ps[:, :n],
                                   wic_s[l][:, k, 128 * m:128 * (m + 1)],
                                   det_sb[l + 1][:, k, n0:n1],
                                   start=(k == 0), stop=(k == 3))
                        nc.vector.tensor_scalar_add(
                            ctxb_sb[l][:, m, n0:n1], ps[:, :n],
                            binc_s[l][:, m:m + 1])

            def scan(l):
                T = TS[l]
                h_f = state.tile([128, 4, b], F32, tag=f"h_f{l}")
                h_b = state.tile([128, 4, b], BF16, tag=f"h_b{l}")
                s_b = state.tile([STOCH, b], BF16, tag=f"s_b{l}")
                nc.vector.memset(h_f, 0.0)
                nc.vector.memset(h_b, 0.0)
                nc.vector.memset(s_b, 0.0)
                n_iter = T // U

                with tc.For_i(0, n_iter, hint_engines=(mybir.EngineType.PE,)) as it:
                    op_sb = gio.tile([128, 4, U * b], BF16, tag="op")
                    nc.sync.dma_start(
                        out=op_sb,
                        in_=obspre[l][:, :, ds(it * (U * b), U * b)]
                        .rearrange("k p n -> p k n"))
                    if l == 0:
                        stage = opool.tile([128, 4, U * b], F32, tag="stage")
                    for u in range(U):
                        sl = slice(u * b, (u + 1) * b)
                        op = op_sb[:, :, sl]
                        if l == 2:
                            cb = cbtop_s[:, :, :]
                        else:
                            cb = ctxb_sb[l][:, :, ds(it * (U * b // 4) + (u // 4) * b, b)]

                        # u = s @ W_in_s ; x = relu(u + ctxb)
                        ps_u = pss.tile([128, 4, b], F32, tag="ps_u")
                        for q in range(4):
                            dep.mm(nc, "ps_u", ps_u[:, q, :],
                                   wis_s[l][:, 128 * q:128 * (q + 1)], s_b,
                                   start=True, stop=True)
                        x_f = work.tile([128, 4, b], F32, tag="x_f")
                        nc.vector.scalar_tensor_tensor(
                            out=x_f, in0=ps_u, scalar=0.0, in1=cb,
                            op0=ALU.bypass, op1=ALU.add)
                        x_b = work.tile([128, 4, b], BF16, tag="x_b")
                        nc.scalar.activation(x_b, x_f, AF.Relu)

                        # gates r,z: (x@Wih + h@Whh)[:, :1024]
                        ps_rz = pss.tile([128, 8, b], F32, tag="ps_rz")
                        for m in range(8):
                            for kk in range(8):
                                k = kk % 4
                                w = wih_s[l] if kk < 4 else whh_s[l]
                                r = x_b if kk < 4 else h_b
                                dep.mm(nc, "ps_rz", ps_rz[:, m, :],
                                       w[:, k, 128 * m:128 * (m + 1)], r[:, k, :],
                                       start=(kk == 0), stop=(kk == 7))
                        # n-gate halves
                        ps_ni = pss.tile([128, 4, b], F32, tag="ps_ni")
                        ps_nh = pss.tile([128, 4, b], F32, tag="ps_nh")
                        for m in range(4):
                            for k in range(4):
                                dep.mm(nc, "ps_nh", ps_nh[:, m, :],
                                       whh_s[l][:, k, 128 * (8 + m):128 * (9 + m)],
                                       h_b[:, k, :], start=(k == 0), stop=(k == 3))
                        for m in range(4):
                            for k in range(4):
                                dep.mm(nc, "ps_ni", ps_ni[:, m, :],
                                       wih_s[l][:, k, 128 * (8 + m):128 * (9 + m)],
                                       x_b[:, k, :], start=(k == 0), stop=(k == 3))

                        rz_f = work.tile([128, 8, b], F32, tag="rz_f")
                        nc.vector.tensor_add(rz_f, ps_rz, brz_s[l])
                        rz_s = work.tile([128, 8, b], F32, tag="rz_s")
                        nc.scalar.activation(rz_s, rz_f, AF.Sigmoid)
                        nh_f = work.tile([128, 4, b], F32, tag="nh_f")
                        nc.vector.tensor_add(nh_f, ps_nh, bhn_s[l])
                        rn = work.tile([128, 4, b], F32, tag="rn")
                        nc.vector.tensor_mul(rn, rz_s[:, 0:4, :], nh_f)
                        ni_f = work.tile([128, 4, b], F32, tag="ni_f")
                        nc.vector.tensor_add(ni_f, ps_ni, binn_s[l])
                        npre = work.tile([128, 4, b], F32, tag="npre")
                        nc.vector.tensor_add(npre, ni_f, rn)
                        n_s = work.tile([128, 4, b], F32, tag="n_s")
                        nc.scalar.activation(n_s, npre, AF.Tanh)

                        # h = n + z*(h - n)
                        d_f = work.tile([128, 4, b], F32, tag="d_f")
                        nc.vector.tensor_sub(d_f, h_f, n_s)
                        zd = work.tile([128, 4, b], F32, tag="zd")
                        nc.vector.tensor_mul(zd, rz_s[:, 4:8, :], d_f)
                        nc.vector.tensor_add(h_f, n_s, zd)
                        nc.scalar.activation(h_b, h_f, AF.Copy)

                        if l == 0:
                            nc.vector.tensor_copy(stage[:, :, sl], h_f)
                        else:
                            nc.vector.tensor_copy(
                                det_sb[l][:, :, ds(it * (U * b) + u * b, b)], h_b)

                        # hx = relu(h @ W_obs_h + obs_pre); s' = hx @ W_post_mean
                        ps_ho = pss.tile([128, 4, b], F32, tag="ps_ho")
                        for m in range(4):
                            for k in range(4):
                                dep.mm(nc, "ps_ho", ps_ho[:, m, :],
                                       woh_s[l][:, k, 128 * m:128 * (m + 1)],
                                       h_b[:, k, :], start=(k == 0), stop=(k == 3))
                        hx_f = work.tile([128, 4, b], F32, tag="hx_f")
                        nc.vector.scalar_tensor_tensor(
                            out=hx_f, in0=ps_ho, scalar=0.0, in1=op,
                            op0=ALU.bypass, op1=ALU.add)
                        hx_b = work.tile([128, 4, b], BF16, tag="hx_b")
                        nc.scalar.activation(hx_b, hx_f, AF.Relu)
                        ps_q = pss.tile([STOCH, b], F32, tag="ps_q")
                        for k in range(4):
                            dep.mm(nc, "ps_q", ps_q,
                                   wpm_s[l][:, k, :STOCH], hx_b[:, k, :],
                                   start=(k == 0), stop=(k == 3))
                        nc.vector.tensor_scalar_add(s_b, ps_q, bpost_s[l])

                    if l == 0:
                        nc.sync.dma_start(
                            out=det0T[:, :, ds(it * (U * b), U * b)]
                            .rearrange("k p n -> p k n"),
                            in_=stage)

            ctx_enabled = do_scan if do_ctx is None else do_ctx
            for l in levels:
                if do_gemm:
                    obs_gemm(l)
                if l < 2:
                    if ctx_enabled and (l + 1) in levels:
                        ctx_gemm(l)
                    elif do_scan:
                        nc.vector.memset(ctxb_sb[l], 0.0)
                if do_scan:
                    scan(l)
            if not (do_scan and 0 in levels):
                # debug: ensure the output tensor is written
                dbg = gio.tile([128, 4, U * BPC], F32, tag="dbg")
                nc.vector.memset(dbg, 0.0)
                nc.sync.dma_start(
                    out=det0T[:, :, 0:U * BPC].rearrange("k p n -> p k n"), in_=dbg)

    nc.finalize()
    return nc


# ---------------- host-side packing ----------------

def _bf(x):
    return np.ascontiguousarray(x).astype(ml_dtypes.bfloat16)


def _prep_shared(params):
    """Weight tensors shared by all cores."""
    b = BPC
    sh = {}
    for l in range(LEVELS):
        p = {k: np.asarray(v, np.float32) for k, v in params[l].items()}
        sh[f"wis{l}"] = _bf(p["W_in"][:STOCH])
        sh[f"wih{l}"] = _bf(p["W_ih"].reshape(4, 128, 3 * DETER))
        sh[f"whh{l}"] = _bf(p["W_hh"].reshape(4, 128, 3 * DETER))
        sh[f"woh{l}"] = _bf(p["W_obs"][:DETER].reshape(4, 128, DETER))
        sh[f"wpm{l}"] = _bf(p["W_post"][:, :STOCH].reshape(4, 128, STOCH))
        sh[f"woo{l}"] = _bf(p["W_obs"][DETER:].reshape(8, 128, DETER))
        if l < 2:
            sh[f"wic{l}"] = _bf(p["W_in"][STOCH:].reshape(4, 128, EMBED))
        gb = p["b_ih"] + p["b_hh"]
        sh[f"brz{l}"] = np.ascontiguousarray(
            np.broadcast_to(gb[:2 * DETER].reshape(8, 128).T[:, :, None], (128, 8, b)),
            ).astype(np.float32)
        sh[f"bhn{l}"] = np.ascontiguousarray(
            np.broadcast_to(p["b_hh"][2 * DETER:].reshape(4, 128).T[:, :, None], (128, 4, b))
            ).astype(np.float32)
        sh[f"binn{l}"] = np.ascontiguousarray(
            np.broadcast_to(p["b_ih"][2 * DETER:].reshape(4, 128).T[:, :, None], (128, 4, b))
            ).astype(np.float32)
        sh[f"bpost{l}"] = np.ascontiguousarray(p["b_post"][:STOCH, None]).astype(np.float32)
        sh[f"bobs{l}"] = np.ascontiguousarray(p["b_obs"].reshape(4, 128).T).astype(np.float32)
        if l < 2:
            sh[f"binc{l}"] = np.ascontiguousarray(p["b_in"].reshape(4, 128).T).astype(np.float32)
        if l == 2:
            sh["cbtop"] = _bf(np.broadcast_to(
                p["b_in"].reshape(4, 128).T[:, :, None], (128, 4, b)))
    return sh


_NC_CACHE = {}


def kernel(obs_l0, obs_l1, obs_l2, params):
    obs = [np.asarray(o, np.float32) for o in (obs_l0, obs_l1, obs_l2)]
    sh = _prep_shared(params)
    in_maps = []
    for c in range(NCORES):
        m = dict(sh)
        for l in range(LEVELS):
            shard = obs[l][c * BPC:(c + 1) * BPC]        # [b, T, 1024]
            m[f"obsT{l}"] = _bf(shard.transpose(2, 1, 0)  # [1024, T, b]
                                .reshape(8, 128, TS[l] * BPC))
        in_maps.append(m)

    if "nc" not in _NC_CACHE:
        _NC_CACHE["nc"] = build_nc()
    nc = _NC_CACHE["nc"]
    res = run_bass_kernel_spmd(nc, in_maps, core_ids=list(range(NCORES)))
    out = np.zeros((B, T0, DETER), np.float32)
    for c in range(NCORES):
        d = res.results[c]["det0T"]                      # [4, 128, T0*b]
        d = d.reshape(4, 128, T0, BPC).transpose(3, 2, 0, 1).reshape(BPC, T0, DETER)
        out[c * BPC:(c + 1) * BPC] = d
    return out
